# revision 41
# baseline (speedup 1.0000x reference)
"""Trainium2 Bass kernel for CrossMultiheadAttention.

B=4, T=S=1024, E=1024, H=16, D=64. 8 NeuronCores.

Sharding: core c handles (batch b=c//2, T-half th=c%2) -> 512 query rows.
Each core computes k/v projections for its whole batch (duplicated between
the 2 cores sharing a batch), all 16 heads of attention for its queries and
the full output projection for its rows. Output gather is a pure concat.

Schedule (v4): PE-centric pipeline that keeps the tensor engine streaming
continuously (p-state stays at max clock):
  - merged multi-tile DMAs, ordered wq/qin -> wv/vin -> wk/kin -> bias/wo
  - pre-attention: q-proj, v-proj (both halves), k-proj head-pair 0
  - attention hp=0..7 per (j, head): score matmul over a zero-padded
    128-deep kTz stationary (start) + attn-bias accumulation via an
    identity-stationary matmul over the bf16 bias tile (stop; key-padding
    mask folded into the bias on host), then exp directly from PSUM (ACT)
    and attn@v with a fused ones-column denominator. k-proj for hp+1 and
    wave-A out-proj accumulation interleave as PE filler; kTz evacuation
    runs on DVE (tensor_scalar_add with the per-partition k bias).
  - per-hp normalization: denominator rows broadcast via bf16 selector
    matmuls, one DVE reciprocal on the broadcast, multiplies on the
    otherwise-idle GPSIMD engine (all-SBUF).
  - out-proj: wave A (2 tiles) accumulates during attention; wave B
    (6 tiles) runs et0..6 bursts through the final norm, then finishes.
"""
import sys

sys.path.insert(0, "/opt/trn_rl_repo")

import numpy as np
import ml_dtypes

import concourse.bass as bass
import concourse.bacc as bacc
import concourse.tile as tile
from concourse import mybir
from concourse.bass_utils import run_bass_kernel_spmd

F32 = mybir.dt.float32
BF16 = mybir.dt.bfloat16
Act = mybir.ActivationFunctionType
Alu = mybir.AluOpType
NPBF16 = ml_dtypes.bfloat16

B, T, S, E, H, D = 4, 1024, 1024, 1024, 16, 64
HP = H // 2          # head pairs
TS = T // 2          # per-core query rows (t-shard)
ET = E // 128        # 128-row tiles of the embed dim
SCALING = D ** -0.5
MASK_NEG = -10000.0

_CACHE = {}


def build_nc():
    nc = bacc.Bacc("TRN2", target_bir_lowering=False, debug=False, num_devices=8)

    qin_d = nc.dram_tensor("qin", [E, TS], BF16, kind="ExternalInput").ap()
    kin_d = nc.dram_tensor("kin", [E, S], BF16, kind="ExternalInput").ap()
    vin_d = nc.dram_tensor("vin", [E, S], BF16, kind="ExternalInput").ap()
    # bias with key-padding mask folded in, transposed to [H, S, TS], bf16
    bias_d = nc.dram_tensor("biasT", [H, S, TS], BF16, kind="ExternalInput").ap()
    wq_d = nc.dram_tensor("wqt", [E, E], BF16, kind="ExternalInput").ap()
    wk_d = nc.dram_tensor("wkt", [E, E], BF16, kind="ExternalInput").ap()
    wv_d = nc.dram_tensor("wvt", [E, E], BF16, kind="ExternalInput").ap()
    wo_d = nc.dram_tensor("wot", [E, E], BF16, kind="ExternalInput").ap()
    bq_d = nc.dram_tensor("bqs", [128, 8], F32, kind="ExternalInput").ap()
    bk_d = nc.dram_tensor("bks", [128, 8], F32, kind="ExternalInput").ap()
    bv_d = nc.dram_tensor("bvr", [128, E], F32, kind="ExternalInput").ap()
    bo_d = nc.dram_tensor("bor", [128, E], F32, kind="ExternalInput").ap()
    sel_d = nc.dram_tensor("sel2", [128, 128], BF16, kind="ExternalInput").ap()
    id_d = nc.dram_tensor("ident", [128, 128], BF16, kind="ExternalInput").ap()
    out_d = nc.dram_tensor("out", [TS, E], F32, kind="ExternalOutput").ap()

    with tile.TileContext(nc) as tc:
        with tc.tile_pool(name="consts", bufs=1) as consts, \
             tc.tile_pool(name="wpool", bufs=1) as wpool, \
             tc.tile_pool(name="kvin", bufs=1) as kvin, \
             tc.tile_pool(name="persist", bufs=1) as persist, \
             tc.tile_pool(name="estream", bufs=6) as estream, \
             tc.tile_pool(name="btp", bufs=8) as btp, \
             tc.tile_pool(name="rcpp", bufs=1) as rcpp, \
             tc.tile_pool(name="osbp", bufs=2) as osbp, \
             tc.tile_pool(name="psmain", bufs=3, space="PSUM") as psmain, \
             tc.tile_pool(name="psk", bufs=1, space="PSUM") as psk, \
             tc.tile_pool(name="psA", bufs=2, space="PSUM") as psAp, \
             tc.tile_pool(name="psot", bufs=1, space="PSUM") as psot:

            # ---- tiny constants first ----
            bq_sb = consts.tile([128, 8], F32, tag="bq")
            bk_sb = consts.tile([128, 8], F32, tag="bk")
            sel_sb = consts.tile([128, 128], BF16, tag="sel")
            id_sb = consts.tile([128, 128], BF16, tag="id")
            ones_col = consts.tile([128, 16], BF16, tag="ones_col")
            bvb = consts.tile([128, E], F32, tag="bvb")
            bob = consts.tile([128, E], F32, tag="bob")
            nc.vector.memset(ones_col, 1.0)

            # ---- input tiles: 2 et-chunks of 4 each (separate tiles so the
            # first chunk's consumers don't wait on the second DMA) ----
            def chunk_tiles(tag, width):
                return [wpool.tile([128, 4, width], BF16, tag=f"{tag}{c}",
                                   name=f"{tag}{c}") for c in range(2)]

            wq_c4 = [wpool.tile([128, 2, E], BF16, tag=f"wqf{c}",
                                name=f"wqf{c}") for c in range(4)]
            wk_c = chunk_tiles("wk", E)
            wo_c = chunk_tiles("wo", E)
            wv_c = [[wpool.tile([128, 4, 512], BF16, tag=f"wv{i}{c}",
                                name=f"wv{i}{c}") for c in range(2)]
                    for i in range(2)]
            qin_c4 = [kvin.tile([128, 2, TS], BF16, tag=f"qinf{c}",
                                name=f"qinf{c}") for c in range(4)]
            kin_c = [kvin.tile([128, 4, S], BF16, tag=f"kin{c}",
                               name=f"kin{c}") for c in range(2)]
            vin_c4 = [kvin.tile([128, 2, S], BF16, tag=f"vinf{c}",
                                name=f"vinf{c}") for c in range(4)]

            def w(tiles, et, colsl):
                return tiles[et // 4][:, et % 4, colsl]

            def load_chunk(dst, src, c, colsl=None):
                # DRAM rows [c*512:(c+1)*512] of src -> dst [128, 4, width]
                if colsl is None:
                    nc.sync.dma_start(
                        out=dst,
                        in_=src[c * 512:(c + 1) * 512, :].rearrange(
                            "(et p) x -> p et x", p=128))
                else:
                    nc.sync.dma_start(
                        out=dst,
                        in_=src[c * 512:(c + 1) * 512, colsl].rearrange(
                            "(et p) x -> p et x", p=128))

            def load_chunk2(dst, src, c, eng=None):
                (eng or nc.sync).dma_start(
                    out=dst,
                    in_=src[c * 256:(c + 1) * 256, :].rearrange(
                        "(et p) x -> p et x", p=128))

            # startup-ordered DMAs: q deps, then v deps, then k deps.
            # qin chunks + small constants issue from the idle GPSIMD queue
            # (SWDGE) so the sync queue's 565ns-per-issue serialization does
            # not delay the weight chunks.
            for c in range(4):
                load_chunk2(wq_c4[c], wq_d, c)
                load_chunk2(qin_c4[c], qin_d, c, eng=nc.gpsimd)
            nc.gpsimd.dma_start(out=bq_sb, in_=bq_d)
            nc.gpsimd.dma_start(out=bk_sb, in_=bk_d)
            nc.gpsimd.dma_start(out=bvb, in_=bv_d)

            def load_wv(i, c):
                nc.sync.dma_start(
                    out=wv_c[i][c],
                    in_=wv_d[c * 512:(c + 1) * 512, i * 512:(i + 1) * 512]
                    .rearrange("(et p) x -> p et x", p=128))

            load_wv(0, 0)
            load_chunk2(vin_c4[0], vin_d, 0)
            load_chunk2(vin_c4[1], vin_d, 1)
            load_wv(0, 1)
            load_chunk2(vin_c4[2], vin_d, 2)
            load_chunk2(vin_c4[3], vin_d, 3)
            load_wv(1, 0)
            load_wv(1, 1)
            nc.gpsimd.dma_start(out=sel_sb, in_=sel_d)
            nc.gpsimd.dma_start(out=id_sb, in_=id_d)
            load_chunk(wk_c[0], wk_d, 0)
            load_chunk(kin_c[0], kin_d, 0)
            load_chunk(wk_c[1], wk_d, 1)
            load_chunk(kin_c[1], kin_d, 1)

            # bias chunk stream: per (hp, hh) two chunks of 4 j-tiles
            bias_tiles = {}

            def load_bias_hp(hp):
                for hh in range(2):
                    h = 2 * hp + hh
                    for half in range(2):
                        t_ = btp.tile([128, 4, TS], BF16, tag="bt", name="bt")
                        nc.sync.dma_start(
                            out=t_,
                            in_=bias_d[h, half * 512:(half + 1) * 512, :]
                            .rearrange("(j p) t -> p j t", p=128))
                        bias_tiles[(hp, hh, half)] = t_

            load_bias_hp(0)
            load_bias_hp(1)
            nc.sync.dma_start(out=bob, in_=bo_d)
            load_chunk(wo_c[0], wo_d, 0)
            load_chunk(wo_c[1], wo_d, 1)

            # ---- persistent products ----
            qT = [persist.tile([128, TS], BF16, tag=f"qt{hp}", name=f"qt{hp}")
                  for hp in range(HP)]
            # zero-padded k tiles: kTz[(sh, hh)] has head hh's rows in
            # partitions hh*64..hh*64+63 and zeros elsewhere, so the score
            # matmul can use a full 128-deep stationary (accumulation-group
            # compatible with the identity bias matmul).
            kTz = {}
            for sh in range(2):
                for hh in range(2):
                    t_ = persist.tile([128, 512], BF16, tag=f"ktz{sh}{hh}",
                                      name=f"ktz{sh}{hh}")
                    nc.vector.memset(t_, 0.0)
                    kTz[(sh, hh)] = t_
            v65 = [persist.tile([128, H, 65], BF16, tag=f"v65_{j}",
                                name=f"v65_{j}")
                   for j in range(8)]
            otn2 = [persist.tile([128, TS], BF16, tag=f"otn2_{et}",
                                 name=f"otn2_{et}")
                    for et in range(ET)]

            # ---- q projection: qT[hp] = wqT.T @ queryT (+ bq, pre-scaled)
            # et-outer over 4-hp halves so the PE consumes each wq/qin chunk
            # as soon as its DMA lands ----
            for half in range(2):
                ps4 = [psmain.tile([128, 512], F32, tag="main", name="psm")
                       for _ in range(3)]
                ps4.append(psk.tile([128, 512], F32, tag="psk", name="psk"))
                for et in range(ET):
                    for i in range(4):
                        hp = half * 4 + i
                        nc.tensor.matmul(
                            ps4[i],
                            wq_c4[et // 2][:, et % 2, hp * 128:(hp + 1) * 128],
                            qin_c4[et // 2][:, et % 2, :],
                            start=(et == 0), stop=(et == ET - 1))
                for i in range(4):
                    hp = half * 4 + i
                    nc.scalar.activation(qT[hp], ps4[i], Act.Identity,
                                         bias=bq_sb[:, hp:hp + 1])

            # ---- v projection into v65 (s-major, per-head 65th col = 1) ----
            for j in range(8):
                nc.vector.tensor_copy(
                    out=v65[j][:, :, 64:65],
                    in_=ones_col.rearrange("p (h o) -> p h o", o=1))
            for ih in range(2):
                for sh in range(2):
                    for sp in range(2):
                        ps2 = [psmain.tile([128, 512], F32, tag="main",
                                           name="psm") for _ in range(2)]
                        for et in range(ET):
                            for i in range(2):
                                st = sp * 2 + i
                                nc.tensor.matmul(
                                    ps2[i],
                                    vin_c4[et // 2][:, et % 2,
                                                    sh * 512 + st * 128:
                                                    sh * 512 + (st + 1) * 128],
                                    wv_c[ih][et // 4][:, et % 4, :],
                                    start=(et == 0), stop=(et == ET - 1))
                        for i in range(2):
                            st = sp * 2 + i
                            j = sh * 4 + st
                            nc.vector.tensor_tensor(
                                out=v65[j][:, ih * 8:(ih + 1) * 8, 0:64],
                                in0=ps2[i].rearrange("p (h d) -> p h d", h=8),
                                in1=bvb[:, ih * 512:(ih + 1) * 512].rearrange(
                                    "p (h d) -> p h d", h=8),
                                op=Alu.add)

            # ---- k projection for head pair hp (inline or as PE filler) ----
            def k_proj(hp):
                items = []
                for sh in range(2):
                    ps_box = {}

                    def mm(et, sh=sh, ps_box=ps_box):
                        if et == 0:
                            ps_box["ps"] = psk.tile(
                                [128, 512], F32, tag="psk", name="psk")
                        nc.tensor.matmul(
                            ps_box["ps"],
                            w(wk_c, et, slice(hp * 128, (hp + 1) * 128)),
                            w(kin_c, et, slice(sh * 512, (sh + 1) * 512)),
                            start=(et == 0), stop=(et == ET - 1))
                        if et == ET - 1:
                            # evacuate into the zero-padded kTz tiles on DVE
                            for hh in range(2):
                                nc.vector.tensor_scalar_add(
                                    out=kTz[(sh, hh)][hh * 64:(hh + 1) * 64, :],
                                    in0=ps_box["ps"][hh * 64:(hh + 1) * 64, :],
                                    scalar1=bk_sb[hh * 64:(hh + 1) * 64,
                                                  hp:hp + 1])
                    items.extend([lambda et=et, mm=mm: mm(et)
                                  for et in range(ET)])
                return items

            # k-proj for hp 0 runs before attention
            for it in k_proj(0):
                it()

            # ---- wave-A out-proj accumulators: tiles (tt=0, ih=0/1) ----
            waveA = [(0, 0), (0, 1)]
            waveB = [(1, 0), (1, 1), (2, 0), (2, 1), (3, 0), (3, 1)]
            wa_ps = {}
            for tt, ih in waveA:
                wa_ps[(tt, ih)] = psAp.tile([128, 512], F32, tag="psA",
                                            name="psA")

            def wa_accum(hp):
                for tt, ih in waveA:
                    nc.tensor.matmul(
                        wa_ps[(tt, ih)],
                        otn2[hp][:, tt * 128:(tt + 1) * 128],
                        w(wo_c, hp, slice(ih * 512, (ih + 1) * 512)),
                        start=(hp == 0), stop=(hp == ET - 1))

            # ---- PE filler queue for the attention phase ----
            filler = []
            for hpx in range(1, HP):
                filler.extend(k_proj(hpx))

            def pull(n):
                for _ in range(n):
                    if filler:
                        filler.pop(0)()

            # ---- attention ----
            for hp in range(HP):
                poT = [psot.tile([65, 512], F32, tag=f"ot{hh}", name=f"ot{hh}")
                       for hh in range(2)]
                e_tiles = {}

                def score_pair(j, hp=hp, e_tiles=e_tiles):
                    sh, sl = j // 4, j % 4
                    pss = []
                    for hh in range(2):
                        pss.append(psmain.tile([128, 512], F32, tag="main",
                                               name="psm"))
                    for hh in range(2):
                        nc.tensor.matmul(
                            pss[hh],
                            kTz[(sh, hh)][:, sl * 128:(sl + 1) * 128],
                            qT[hp], start=True, stop=False)
                        bt = bias_tiles[(hp, hh, j // 4)]
                        nc.tensor.matmul(
                            pss[hh], id_sb, bt[:, j % 4, :],
                            start=False, stop=True)
                    pull(2)
                    for hh in range(2):
                        e_ = estream.tile([128, TS], BF16, tag="e", name="e")
                        nc.scalar.activation(e_, pss[hh], Act.Exp)
                        e_tiles[(j, hh)] = e_

                def o_mm(j, hh, hp=hp, poT=poT, e_tiles=e_tiles):
                    h = 2 * hp + hh
                    nc.tensor.matmul(poT[hh], v65[j][:, h, :],
                                     e_tiles.pop((j, hh)),
                                     start=(j == 0), stop=(j == 7))

                for j in range(8):
                    score_pair(j)
                    if j >= 2:
                        for hh in range(2):
                            o_mm(j - 2, hh)
                for jj in (6, 7):
                    for hh in range(2):
                        o_mm(jj, hh)

                # prefetch bias for hp+2
                if hp + 2 < HP:
                    load_bias_hp(hp + 2)

                # ---- per-hp normalization ----
                den2 = rcpp.tile([128, 512], BF16, tag="den", name="den")
                ot_sb = rcpp.tile([128, 512], F32, tag="ots", name="ots")
                for hh in range(2):
                    nc.vector.tensor_copy(out=den2[64 * hh:64 * hh + 1, :],
                                          in_=poT[hh][64:65, :])
                    if hp < 7:
                        # evacuate early so poT's last reader is cheap (the
                        # next hp's o_mm WARs it); hp7 multiplies from PSUM
                        nc.vector.tensor_copy(
                            out=ot_sb[64 * hh:64 * (hh + 1), :],
                            in_=poT[hh][0:64, :])
                # wave-A accumulation for the previous hp covers the copy
                # latency (its otn2 is a full window old - no stall)
                if hp >= 1:
                    wa_accum(hp - 1)
                bc = psmain.tile([128, 512], F32, tag="main", name="psm")
                for hh in range(2):
                    # independent groups with disjoint 64-partition regions
                    nc.tensor.matmul(
                        bc[64 * hh:64 * (hh + 1), :],
                        sel_sb[64 * hh:64 * hh + 1, 64 * hh:64 * (hh + 1)],
                        den2[64 * hh:64 * hh + 1, :],
                        start=True, stop=True)
                bc_sb = rcpp.tile([128, 512], F32, tag="bcs", name="bcs")
                nc.vector.tensor_copy(out=bc_sb, in_=bc)
                rcp_sb = rcpp.tile([128, 512], F32, tag="rcps", name="rcps")
                nc.vector.reciprocal(out=rcp_sb, in_=bc_sb)
                for hh in range(2):
                    if hp < 7:
                        # all-SBUF multiply on the otherwise-idle GPSIMD
                        nc.gpsimd.tensor_tensor(
                            out=otn2[hp][hh * 64:(hh + 1) * 64, :],
                            in0=ot_sb[hh * 64:(hh + 1) * 64, :],
                            in1=rcp_sb[hh * 64:(hh + 1) * 64, :],
                            op=Alu.mult)
                    else:
                        nc.vector.tensor_tensor(
                            out=otn2[hp][hh * 64:(hh + 1) * 64, :],
                            in0=poT[hh][0:64, :],
                            in1=rcp_sb[hh * 64:(hh + 1) * 64, :],
                            op=Alu.mult)


            # ---- output projection tail ----
            # wave-B et0..6 bursts keep the PE hot while norm(7) finishes;
            # the et==7 matmul + evacuation follow once otn2[7] lands.
            wb_ps = {}

            def wb_burst(tt, ih, pool=None):
                wb_ps[(tt, ih)] = (pool or psmain).tile(
                    [128, 512], F32,
                    tag="main" if pool is None else "psk", name="psm")
                for et in range(ET - 1):
                    nc.tensor.matmul(
                        wb_ps[(tt, ih)],
                        otn2[et][:, tt * 128:(tt + 1) * 128],
                        w(wo_c, et, slice(ih * 512, (ih + 1) * 512)),
                        start=(et == 0), stop=False)

            def evac(ps, tt, ih):
                o = osbp.tile([128, 512], F32, tag="osb", name="osb")
                nc.vector.tensor_tensor(
                    out=o, in0=ps, in1=bob[:, ih * 512:(ih + 1) * 512],
                    op=Alu.add)
                nc.sync.dma_start(
                    out=out_d[tt * 128:(tt + 1) * 128,
                              ih * 512:(ih + 1) * 512],
                    in_=o)

            def wb_finish(tt, ih):
                nc.tensor.matmul(
                    wb_ps[(tt, ih)],
                    otn2[7][:, tt * 128:(tt + 1) * 128],
                    w(wo_c, 7, slice(ih * 512, (ih + 1) * 512)),
                    start=False, stop=True)
                evac(wb_ps[(tt, ih)], tt, ih)

            g1, g2 = waveB[:4], waveB[4:]
            for i, (tt, ih) in enumerate(g1):
                wb_burst(tt, ih, pool=psk if i == 3 else None)
            # final wave-A accumulation (otn2[7] lands during the bursts)
            wa_accum(7)
            while filler:
                filler.pop(0)()
            for tt, ih in waveA:
                evac(wa_ps[(tt, ih)], tt, ih)
            for tt, ih in g1:
                wb_finish(tt, ih)
            for tt, ih in g2:
                wb_burst(tt, ih)
            for tt, ih in g2:
                wb_finish(tt, ih)

    nc.compile()
    return nc


def _prepare_in_maps(query, key, value, key_padding_mask, attn_bias,
                     wq, bq, wk, bk, wv, bv, wo, bo):
    wqt = (np.ascontiguousarray(wq.T) * SCALING).astype(NPBF16)
    wkt = np.ascontiguousarray(wk.T).astype(NPBF16)
    wvt = np.ascontiguousarray(wv.T).astype(NPBF16)
    wot = np.ascontiguousarray(wo.T).astype(NPBF16)
    bqs = np.ascontiguousarray((bq * SCALING).reshape(8, 128).T)
    bks = np.ascontiguousarray(bk.astype(np.float32).reshape(8, 128).T)
    bvr = np.ascontiguousarray(np.broadcast_to(
        np.asarray(bv, np.float32)[None, :], (128, E)))
    bor = np.ascontiguousarray(np.broadcast_to(
        np.asarray(bo, np.float32)[None, :], (128, E)))
    sel2 = np.zeros((128, 128), NPBF16)
    sel2[0, :64] = 1.0
    sel2[64, 64:] = 1.0
    ident = np.eye(128, dtype=NPBF16)

    kin_b = [np.ascontiguousarray(key[b_].T).astype(NPBF16) for b_ in range(B)]
    vin_b = [np.ascontiguousarray(value[b_].T).astype(NPBF16) for b_ in range(B)]
    masked = [np.where(key_padding_mask[b_], np.float32(MASK_NEG),
                       np.float32(0.0)) for b_ in range(B)]

    in_maps = []
    for c in range(8):
        b_, th = c // 2, c % 2
        qin = np.ascontiguousarray(
            query[b_, th * TS:(th + 1) * TS, :].T).astype(NPBF16)
        biasT = (attn_bias[b_ * H:(b_ + 1) * H, th * TS:(th + 1) * TS, :]
                 .transpose(0, 2, 1) + masked[b_][None, :, None])
        biasT = np.ascontiguousarray(biasT).astype(NPBF16)
        in_maps.append({
            "qin": qin, "kin": kin_b[b_], "vin": vin_b[b_],
            "biasT": biasT,
            "wqt": wqt, "wkt": wkt, "wvt": wvt, "wot": wot,
            "bqs": bqs, "bks": bks, "bvr": bvr, "bor": bor,
            "sel2": sel2, "ident": ident,
        })
    return in_maps


def kernel(query, key, value, key_padding_mask, attn_bias,
           wq, bq, wk, bk, wv, bv, wo, bo, _run_kwargs=None):
    query = np.asarray(query, dtype=np.float32)
    key = np.asarray(key, dtype=np.float32)
    value = np.asarray(value, dtype=np.float32)
    key_padding_mask = np.asarray(key_padding_mask)
    attn_bias = np.asarray(attn_bias, dtype=np.float32)
    wq, bq = np.asarray(wq, np.float32), np.asarray(bq, np.float32)
    wk, bk = np.asarray(wk, np.float32), np.asarray(bk, np.float32)
    wv, bv = np.asarray(wv, np.float32), np.asarray(bv, np.float32)
    wo, bo = np.asarray(wo, np.float32), np.asarray(bo, np.float32)

    if "nc" not in _CACHE:
        _CACHE["nc"] = build_nc()
    nc = _CACHE["nc"]

    in_maps = _prepare_in_maps(query, key, value, key_padding_mask, attn_bias,
                               wq, bq, wk, bk, wv, bv, wo, bo)
    res = run_bass_kernel_spmd(nc, in_maps, core_ids=list(range(8)),
                               **(_run_kwargs or {}))
    _CACHE["last_results"] = res

    out = np.empty((B, T, E), dtype=np.float32)
    for c in range(8):
        b_, th = c // 2, c % 2
        out[b_, th * TS:(th + 1) * TS, :] = res.results[c]["out"]
    return out


# revision 42
# speedup vs baseline: 1.0054x; 1.0054x over previous
"""Trainium2 Bass kernel for CrossMultiheadAttention.

B=4, T=S=1024, E=1024, H=16, D=64. 8 NeuronCores.

Sharding: core c handles (batch b=c//2, T-half th=c%2) -> 512 query rows.
Each core computes k/v projections for its whole batch (duplicated between
the 2 cores sharing a batch), all 16 heads of attention for its queries and
the full output projection for its rows. Output gather is a pure concat.

Schedule (v4): PE-centric pipeline that keeps the tensor engine streaming
continuously (p-state stays at max clock):
  - merged multi-tile DMAs, ordered wq/qin -> wv/vin -> wk/kin -> bias/wo
  - pre-attention: q-proj, v-proj (both halves), k-proj head-pair 0
  - attention hp=0..7 per (j, head): score matmul over a zero-padded
    128-deep kTz stationary (start) + attn-bias accumulation via an
    identity-stationary matmul over the bf16 bias tile (stop; key-padding
    mask folded into the bias on host), then exp directly from PSUM (ACT)
    and attn@v with a fused ones-column denominator. k-proj for hp+1 and
    wave-A out-proj accumulation interleave as PE filler; kTz evacuation
    runs on DVE (tensor_scalar_add with the per-partition k bias).
  - per-hp normalization: denominator rows broadcast via bf16 selector
    matmuls, one DVE reciprocal on the broadcast, multiplies on the
    otherwise-idle GPSIMD engine (all-SBUF).
  - out-proj: wave A (2 tiles) accumulates during attention; wave B
    (6 tiles) runs et0..6 bursts through the final norm, then finishes.
"""
import sys

sys.path.insert(0, "/opt/trn_rl_repo")

import numpy as np
import ml_dtypes

import concourse.bass as bass
import concourse.bacc as bacc
import concourse.tile as tile
from concourse import mybir
from concourse.bass_utils import run_bass_kernel_spmd

F32 = mybir.dt.float32
BF16 = mybir.dt.bfloat16
Act = mybir.ActivationFunctionType
Alu = mybir.AluOpType
NPBF16 = ml_dtypes.bfloat16

B, T, S, E, H, D = 4, 1024, 1024, 1024, 16, 64
HP = H // 2          # head pairs
TS = T // 2          # per-core query rows (t-shard)
ET = E // 128        # 128-row tiles of the embed dim
SCALING = D ** -0.5
MASK_NEG = -10000.0

_CACHE = {}


def build_nc():
    nc = bacc.Bacc("TRN2", target_bir_lowering=False, debug=False, num_devices=8)

    qin_d = nc.dram_tensor("qin", [E, TS], BF16, kind="ExternalInput").ap()
    kin_d = nc.dram_tensor("kin", [E, S], BF16, kind="ExternalInput").ap()
    vin_d = nc.dram_tensor("vin", [E, S], BF16, kind="ExternalInput").ap()
    # bias with key-padding mask folded in, transposed to [H, S, TS], bf16
    bias_d = nc.dram_tensor("biasT", [H, S, TS], BF16, kind="ExternalInput").ap()
    wq_d = nc.dram_tensor("wqt", [E, E], BF16, kind="ExternalInput").ap()
    wk_d = nc.dram_tensor("wkt", [E, E], BF16, kind="ExternalInput").ap()
    wv_d = nc.dram_tensor("wvt", [E, E], BF16, kind="ExternalInput").ap()
    wo_d = nc.dram_tensor("wot", [E, E], BF16, kind="ExternalInput").ap()
    bq_d = nc.dram_tensor("bqs", [128, 8], F32, kind="ExternalInput").ap()
    bk_d = nc.dram_tensor("bks", [128, 8], F32, kind="ExternalInput").ap()
    bv_d = nc.dram_tensor("bvr", [128, E], F32, kind="ExternalInput").ap()
    bo_d = nc.dram_tensor("bor", [128, E], F32, kind="ExternalInput").ap()
    sel_d = nc.dram_tensor("sel2", [128, 128], BF16, kind="ExternalInput").ap()
    id_d = nc.dram_tensor("ident", [128, 128], BF16, kind="ExternalInput").ap()
    out_d = nc.dram_tensor("out", [TS, E], F32, kind="ExternalOutput").ap()

    with tile.TileContext(nc) as tc:
        with tc.tile_pool(name="consts", bufs=1) as consts, \
             tc.tile_pool(name="wpool", bufs=1) as wpool, \
             tc.tile_pool(name="kvin", bufs=1) as kvin, \
             tc.tile_pool(name="persist", bufs=1) as persist, \
             tc.tile_pool(name="estream", bufs=6) as estream, \
             tc.tile_pool(name="btp", bufs=8) as btp, \
             tc.tile_pool(name="rcpp", bufs=1) as rcpp, \
             tc.tile_pool(name="osbp", bufs=2) as osbp, \
             tc.tile_pool(name="psmain", bufs=3, space="PSUM") as psmain, \
             tc.tile_pool(name="psk", bufs=1, space="PSUM") as psk, \
             tc.tile_pool(name="psA", bufs=2, space="PSUM") as psAp, \
             tc.tile_pool(name="psot", bufs=1, space="PSUM") as psot:

            # ---- tiny constants first ----
            bq_sb = consts.tile([128, 8], F32, tag="bq")
            bk_sb = consts.tile([128, 8], F32, tag="bk")
            sel_sb = consts.tile([128, 128], BF16, tag="sel")
            id_sb = consts.tile([128, 128], BF16, tag="id")
            ones_col = consts.tile([128, 16], BF16, tag="ones_col")
            bvb = consts.tile([128, E], F32, tag="bvb")
            bob = consts.tile([128, E], F32, tag="bob")
            nc.vector.memset(ones_col, 1.0)

            # ---- input tiles: 2 et-chunks of 4 each (separate tiles so the
            # first chunk's consumers don't wait on the second DMA) ----
            def chunk_tiles(tag, width):
                return [wpool.tile([128, 4, width], BF16, tag=f"{tag}{c}",
                                   name=f"{tag}{c}") for c in range(2)]

            wq_c4 = [wpool.tile([128, 2, E], BF16, tag=f"wqf{c}",
                                name=f"wqf{c}") for c in range(4)]
            wk_c = chunk_tiles("wk", E)
            wo_c = chunk_tiles("wo", E)
            wv_c = [[wpool.tile([128, 4, 512], BF16, tag=f"wv{i}{c}",
                                name=f"wv{i}{c}") for c in range(2)]
                    for i in range(2)]
            qin_c4 = [kvin.tile([128, 2, TS], BF16, tag=f"qinf{c}",
                                name=f"qinf{c}") for c in range(4)]
            kin_c = [kvin.tile([128, 4, S], BF16, tag=f"kin{c}",
                               name=f"kin{c}") for c in range(2)]
            vin_c4 = [kvin.tile([128, 2, S], BF16, tag=f"vinf{c}",
                                name=f"vinf{c}") for c in range(4)]

            def w(tiles, et, colsl):
                return tiles[et // 4][:, et % 4, colsl]

            def load_chunk(dst, src, c, colsl=None):
                # DRAM rows [c*512:(c+1)*512] of src -> dst [128, 4, width]
                if colsl is None:
                    nc.sync.dma_start(
                        out=dst,
                        in_=src[c * 512:(c + 1) * 512, :].rearrange(
                            "(et p) x -> p et x", p=128))
                else:
                    nc.sync.dma_start(
                        out=dst,
                        in_=src[c * 512:(c + 1) * 512, colsl].rearrange(
                            "(et p) x -> p et x", p=128))

            def load_chunk2(dst, src, c):
                nc.sync.dma_start(
                    out=dst,
                    in_=src[c * 256:(c + 1) * 256, :].rearrange(
                        "(et p) x -> p et x", p=128))

            # startup-ordered DMAs: q deps, then v deps, then k deps
            for c in range(4):
                load_chunk2(wq_c4[c], wq_d, c)
                load_chunk2(qin_c4[c], qin_d, c)
            nc.sync.dma_start(out=bq_sb, in_=bq_d)
            nc.sync.dma_start(out=bk_sb, in_=bk_d)
            nc.sync.dma_start(out=bvb, in_=bv_d)

            def load_wv(i, c):
                nc.sync.dma_start(
                    out=wv_c[i][c],
                    in_=wv_d[c * 512:(c + 1) * 512, i * 512:(i + 1) * 512]
                    .rearrange("(et p) x -> p et x", p=128))

            load_wv(0, 0)
            load_chunk2(vin_c4[0], vin_d, 0)
            load_chunk2(vin_c4[1], vin_d, 1)
            load_wv(0, 1)
            load_chunk2(vin_c4[2], vin_d, 2)
            load_chunk2(vin_c4[3], vin_d, 3)
            load_wv(1, 0)
            load_wv(1, 1)
            nc.sync.dma_start(out=sel_sb, in_=sel_d)
            nc.sync.dma_start(out=id_sb, in_=id_d)
            load_chunk(wk_c[0], wk_d, 0)
            load_chunk(kin_c[0], kin_d, 0)
            load_chunk(wk_c[1], wk_d, 1)
            load_chunk(kin_c[1], kin_d, 1)

            # bias chunk stream: per (hp, hh) two chunks of 4 j-tiles
            bias_tiles = {}

            def load_bias_hp(hp):
                for hh in range(2):
                    h = 2 * hp + hh
                    for half in range(2):
                        t_ = btp.tile([128, 4, TS], BF16, tag="bt", name="bt")
                        nc.sync.dma_start(
                            out=t_,
                            in_=bias_d[h, half * 512:(half + 1) * 512, :]
                            .rearrange("(j p) t -> p j t", p=128))
                        bias_tiles[(hp, hh, half)] = t_

            load_bias_hp(0)
            load_bias_hp(1)
            nc.sync.dma_start(out=bob, in_=bo_d)
            load_chunk(wo_c[0], wo_d, 0)
            load_chunk(wo_c[1], wo_d, 1)

            # ---- persistent products ----
            qT = [persist.tile([128, TS], BF16, tag=f"qt{hp}", name=f"qt{hp}")
                  for hp in range(HP)]
            # zero-padded k tiles: kTz[(sh, hh)] has head hh's rows in
            # partitions hh*64..hh*64+63 and zeros elsewhere, so the score
            # matmul can use a full 128-deep stationary (accumulation-group
            # compatible with the identity bias matmul).
            kTz = {}
            for sh in range(2):
                for hh in range(2):
                    t_ = persist.tile([128, 512], BF16, tag=f"ktz{sh}{hh}",
                                      name=f"ktz{sh}{hh}")
                    nc.vector.memset(t_, 0.0)
                    kTz[(sh, hh)] = t_
            v65 = [persist.tile([128, H, 65], BF16, tag=f"v65_{j}",
                                name=f"v65_{j}")
                   for j in range(8)]
            otn2 = [persist.tile([128, TS], BF16, tag=f"otn2_{et}",
                                 name=f"otn2_{et}")
                    for et in range(ET)]

            # ---- q projection: qT[hp] = wqT.T @ queryT (+ bq, pre-scaled)
            # et-outer over 4-hp halves so the PE consumes each wq/qin chunk
            # as soon as its DMA lands ----
            for half in range(2):
                ps4 = [psmain.tile([128, 512], F32, tag="main", name="psm")
                       for _ in range(3)]
                ps4.append(psk.tile([128, 512], F32, tag="psk", name="psk"))
                for et in range(ET):
                    for i in range(4):
                        hp = half * 4 + i
                        nc.tensor.matmul(
                            ps4[i],
                            wq_c4[et // 2][:, et % 2, hp * 128:(hp + 1) * 128],
                            qin_c4[et // 2][:, et % 2, :],
                            start=(et == 0), stop=(et == ET - 1))
                for i in range(4):
                    hp = half * 4 + i
                    nc.scalar.activation(qT[hp], ps4[i], Act.Identity,
                                         bias=bq_sb[:, hp:hp + 1])

            # ---- v projection into v65 (s-major, per-head 65th col = 1) ----
            for j in range(8):
                nc.vector.tensor_copy(
                    out=v65[j][:, :, 64:65],
                    in_=ones_col.rearrange("p (h o) -> p h o", o=1))
            for ih in range(2):
                for sh in range(2):
                    for sp in range(2):
                        ps2 = [psmain.tile([128, 512], F32, tag="main",
                                           name="psm") for _ in range(2)]
                        for et in range(ET):
                            for i in range(2):
                                st = sp * 2 + i
                                nc.tensor.matmul(
                                    ps2[i],
                                    vin_c4[et // 2][:, et % 2,
                                                    sh * 512 + st * 128:
                                                    sh * 512 + (st + 1) * 128],
                                    wv_c[ih][et // 4][:, et % 4, :],
                                    start=(et == 0), stop=(et == ET - 1))
                        for i in range(2):
                            st = sp * 2 + i
                            j = sh * 4 + st
                            nc.vector.tensor_tensor(
                                out=v65[j][:, ih * 8:(ih + 1) * 8, 0:64],
                                in0=ps2[i].rearrange("p (h d) -> p h d", h=8),
                                in1=bvb[:, ih * 512:(ih + 1) * 512].rearrange(
                                    "p (h d) -> p h d", h=8),
                                op=Alu.add)

            # ---- k projection for head pair hp (inline or as PE filler) ----
            def k_proj(hp):
                items = []
                for sh in range(2):
                    ps_box = {}

                    def mm(et, sh=sh, ps_box=ps_box):
                        if et == 0:
                            ps_box["ps"] = psk.tile(
                                [128, 512], F32, tag="psk", name="psk")
                        nc.tensor.matmul(
                            ps_box["ps"],
                            w(wk_c, et, slice(hp * 128, (hp + 1) * 128)),
                            w(kin_c, et, slice(sh * 512, (sh + 1) * 512)),
                            start=(et == 0), stop=(et == ET - 1))
                        if et == ET - 1:
                            # evacuate into the zero-padded kTz tiles on DVE
                            for hh in range(2):
                                nc.vector.tensor_scalar_add(
                                    out=kTz[(sh, hh)][hh * 64:(hh + 1) * 64, :],
                                    in0=ps_box["ps"][hh * 64:(hh + 1) * 64, :],
                                    scalar1=bk_sb[hh * 64:(hh + 1) * 64,
                                                  hp:hp + 1])
                    items.extend([lambda et=et, mm=mm: mm(et)
                                  for et in range(ET)])
                return items

            # k-proj for hp 0 runs before attention
            for it in k_proj(0):
                it()

            # ---- wave-A out-proj accumulators: tiles (tt=0, ih=0/1) ----
            waveA = [(0, 0), (0, 1)]
            waveB = [(1, 0), (1, 1), (2, 0), (2, 1), (3, 0), (3, 1)]
            wa_ps = {}
            for tt, ih in waveA:
                wa_ps[(tt, ih)] = psAp.tile([128, 512], F32, tag="psA",
                                            name="psA")

            def wa_accum(hp):
                for tt, ih in waveA:
                    nc.tensor.matmul(
                        wa_ps[(tt, ih)],
                        otn2[hp][:, tt * 128:(tt + 1) * 128],
                        w(wo_c, hp, slice(ih * 512, (ih + 1) * 512)),
                        start=(hp == 0), stop=(hp == ET - 1))

            # ---- PE filler queue for the attention phase ----
            filler = []
            for hpx in range(1, HP):
                filler.extend(k_proj(hpx))

            def pull(n):
                for _ in range(n):
                    if filler:
                        filler.pop(0)()

            # ---- attention ----
            for hp in range(HP):
                poT = [psot.tile([65, 512], F32, tag=f"ot{hh}", name=f"ot{hh}")
                       for hh in range(2)]
                e_tiles = {}

                def score_pair(j, hp=hp, e_tiles=e_tiles):
                    sh, sl = j // 4, j % 4
                    pss = []
                    for hh in range(2):
                        pss.append(psmain.tile([128, 512], F32, tag="main",
                                               name="psm"))
                    for hh in range(2):
                        nc.tensor.matmul(
                            pss[hh],
                            kTz[(sh, hh)][:, sl * 128:(sl + 1) * 128],
                            qT[hp], start=True, stop=False)
                        bt = bias_tiles[(hp, hh, j // 4)]
                        nc.tensor.matmul(
                            pss[hh], id_sb, bt[:, j % 4, :],
                            start=False, stop=True)
                    pull(2)
                    for hh in range(2):
                        e_ = estream.tile([128, TS], BF16, tag="e", name="e")
                        nc.scalar.activation(e_, pss[hh], Act.Exp)
                        e_tiles[(j, hh)] = e_

                def o_mm(j, hh, hp=hp, poT=poT, e_tiles=e_tiles):
                    h = 2 * hp + hh
                    nc.tensor.matmul(poT[hh], v65[j][:, h, :],
                                     e_tiles.pop((j, hh)),
                                     start=(j == 0), stop=(j == 7))

                for j in range(8):
                    score_pair(j)
                    if j >= 2:
                        for hh in range(2):
                            o_mm(j - 2, hh)
                for jj in (6, 7):
                    for hh in range(2):
                        o_mm(jj, hh)

                # prefetch bias for hp+2
                if hp + 2 < HP:
                    load_bias_hp(hp + 2)

                # ---- per-hp normalization ----
                den2 = rcpp.tile([128, 512], BF16, tag="den", name="den")
                ot_sb = rcpp.tile([128, 512], F32, tag="ots", name="ots")
                for hh in range(2):
                    nc.vector.tensor_copy(out=den2[64 * hh:64 * hh + 1, :],
                                          in_=poT[hh][64:65, :])
                    if hp < 7:
                        # evacuate early so poT's last reader is cheap (the
                        # next hp's o_mm WARs it); hp7 multiplies from PSUM
                        nc.vector.tensor_copy(
                            out=ot_sb[64 * hh:64 * (hh + 1), :],
                            in_=poT[hh][0:64, :])
                # wave-A accumulation for the previous hp covers the copy
                # latency (its otn2 is a full window old - no stall)
                if hp >= 1:
                    wa_accum(hp - 1)
                bc = psmain.tile([128, 512], F32, tag="main", name="psm")
                for hh in range(2):
                    # independent groups with disjoint 64-partition regions
                    nc.tensor.matmul(
                        bc[64 * hh:64 * (hh + 1), :],
                        sel_sb[64 * hh:64 * hh + 1, 64 * hh:64 * (hh + 1)],
                        den2[64 * hh:64 * hh + 1, :],
                        start=True, stop=True)
                bc_sb = rcpp.tile([128, 512], F32, tag="bcs", name="bcs")
                nc.vector.tensor_copy(out=bc_sb, in_=bc)
                rcp_sb = rcpp.tile([128, 512], F32, tag="rcps", name="rcps")
                nc.vector.reciprocal(out=rcp_sb, in_=bc_sb)
                for hh in range(2):
                    if hp < 7:
                        # all-SBUF multiply on the otherwise-idle GPSIMD
                        nc.gpsimd.tensor_tensor(
                            out=otn2[hp][hh * 64:(hh + 1) * 64, :],
                            in0=ot_sb[hh * 64:(hh + 1) * 64, :],
                            in1=rcp_sb[hh * 64:(hh + 1) * 64, :],
                            op=Alu.mult)
                    else:
                        nc.vector.tensor_tensor(
                            out=otn2[hp][hh * 64:(hh + 1) * 64, :],
                            in0=poT[hh][0:64, :],
                            in1=rcp_sb[hh * 64:(hh + 1) * 64, :],
                            op=Alu.mult)


            # ---- output projection tail ----
            # wave-B et0..6 bursts keep the PE hot while norm(7) finishes;
            # the et==7 matmul + evacuation follow once otn2[7] lands.
            wb_ps = {}

            def wb_burst(tt, ih, pool=None):
                wb_ps[(tt, ih)] = (pool or psmain).tile(
                    [128, 512], F32,
                    tag="main" if pool is None else "psk", name="psm")
                for et in range(ET - 1):
                    nc.tensor.matmul(
                        wb_ps[(tt, ih)],
                        otn2[et][:, tt * 128:(tt + 1) * 128],
                        w(wo_c, et, slice(ih * 512, (ih + 1) * 512)),
                        start=(et == 0), stop=False)

            def evac(ps, tt, ih):
                o = osbp.tile([128, 512], F32, tag="osb", name="osb")
                nc.vector.tensor_tensor(
                    out=o, in0=ps, in1=bob[:, ih * 512:(ih + 1) * 512],
                    op=Alu.add)
                nc.sync.dma_start(
                    out=out_d[tt * 128:(tt + 1) * 128,
                              ih * 512:(ih + 1) * 512],
                    in_=o)

            def wb_finish(tt, ih):
                nc.tensor.matmul(
                    wb_ps[(tt, ih)],
                    otn2[7][:, tt * 128:(tt + 1) * 128],
                    w(wo_c, 7, slice(ih * 512, (ih + 1) * 512)),
                    start=False, stop=True)
                evac(wb_ps[(tt, ih)], tt, ih)

            g1, g2 = waveB[:4], waveB[4:]
            for i, (tt, ih) in enumerate(g1):
                wb_burst(tt, ih, pool=psk if i == 3 else None)
            # final wave-A accumulation (otn2[7] lands during the bursts)
            wa_accum(7)
            while filler:
                filler.pop(0)()
            for tt, ih in waveA:
                evac(wa_ps[(tt, ih)], tt, ih)
            for tt, ih in g1:
                wb_finish(tt, ih)
            for tt, ih in g2:
                wb_burst(tt, ih)
            for tt, ih in g2:
                wb_finish(tt, ih)

    nc.compile()
    return nc


def _prepare_in_maps(query, key, value, key_padding_mask, attn_bias,
                     wq, bq, wk, bk, wv, bv, wo, bo):
    wqt = (np.ascontiguousarray(wq.T) * SCALING).astype(NPBF16)
    wkt = np.ascontiguousarray(wk.T).astype(NPBF16)
    wvt = np.ascontiguousarray(wv.T).astype(NPBF16)
    wot = np.ascontiguousarray(wo.T).astype(NPBF16)
    bqs = np.ascontiguousarray((bq * SCALING).reshape(8, 128).T)
    bks = np.ascontiguousarray(bk.astype(np.float32).reshape(8, 128).T)
    bvr = np.ascontiguousarray(np.broadcast_to(
        np.asarray(bv, np.float32)[None, :], (128, E)))
    bor = np.ascontiguousarray(np.broadcast_to(
        np.asarray(bo, np.float32)[None, :], (128, E)))
    sel2 = np.zeros((128, 128), NPBF16)
    sel2[0, :64] = 1.0
    sel2[64, 64:] = 1.0
    ident = np.eye(128, dtype=NPBF16)

    kin_b = [np.ascontiguousarray(key[b_].T).astype(NPBF16) for b_ in range(B)]
    vin_b = [np.ascontiguousarray(value[b_].T).astype(NPBF16) for b_ in range(B)]
    masked = [np.where(key_padding_mask[b_], np.float32(MASK_NEG),
                       np.float32(0.0)) for b_ in range(B)]

    in_maps = []
    for c in range(8):
        b_, th = c // 2, c % 2
        qin = np.ascontiguousarray(
            query[b_, th * TS:(th + 1) * TS, :].T).astype(NPBF16)
        biasT = (attn_bias[b_ * H:(b_ + 1) * H, th * TS:(th + 1) * TS, :]
                 .transpose(0, 2, 1) + masked[b_][None, :, None])
        biasT = np.ascontiguousarray(biasT).astype(NPBF16)
        in_maps.append({
            "qin": qin, "kin": kin_b[b_], "vin": vin_b[b_],
            "biasT": biasT,
            "wqt": wqt, "wkt": wkt, "wvt": wvt, "wot": wot,
            "bqs": bqs, "bks": bks, "bvr": bvr, "bor": bor,
            "sel2": sel2, "ident": ident,
        })
    return in_maps


def kernel(query, key, value, key_padding_mask, attn_bias,
           wq, bq, wk, bk, wv, bv, wo, bo, _run_kwargs=None):
    query = np.asarray(query, dtype=np.float32)
    key = np.asarray(key, dtype=np.float32)
    value = np.asarray(value, dtype=np.float32)
    key_padding_mask = np.asarray(key_padding_mask)
    attn_bias = np.asarray(attn_bias, dtype=np.float32)
    wq, bq = np.asarray(wq, np.float32), np.asarray(bq, np.float32)
    wk, bk = np.asarray(wk, np.float32), np.asarray(bk, np.float32)
    wv, bv = np.asarray(wv, np.float32), np.asarray(bv, np.float32)
    wo, bo = np.asarray(wo, np.float32), np.asarray(bo, np.float32)

    if "nc" not in _CACHE:
        _CACHE["nc"] = build_nc()
    nc = _CACHE["nc"]

    in_maps = _prepare_in_maps(query, key, value, key_padding_mask, attn_bias,
                               wq, bq, wk, bk, wv, bv, wo, bo)
    res = run_bass_kernel_spmd(nc, in_maps, core_ids=list(range(8)),
                               **(_run_kwargs or {}))
    _CACHE["last_results"] = res

    out = np.empty((B, T, E), dtype=np.float32)
    for c in range(8):
        b_, th = c // 2, c % 2
        out[b_, th * TS:(th + 1) * TS, :] = res.results[c]["out"]
    return out


# revision 43
# speedup vs baseline: 1.0078x; 1.0024x over previous
"""Trainium2 Bass kernel for CrossMultiheadAttention.

B=4, T=S=1024, E=1024, H=16, D=64. 8 NeuronCores.

Sharding: core c handles (batch b=c//2, T-half th=c%2) -> 512 query rows.
Each core computes k/v projections for its whole batch (duplicated between
the 2 cores sharing a batch), all 16 heads of attention for its queries and
the full output projection for its rows. Output gather is a pure concat.

Schedule (v4): PE-centric pipeline that keeps the tensor engine streaming
continuously (p-state stays at max clock):
  - merged multi-tile DMAs, ordered wq/qin -> wv/vin -> wk/kin -> bias/wo
  - pre-attention: q-proj, v-proj (both halves), k-proj head-pair 0
  - attention hp=0..7 per (j, head): score matmul over a zero-padded
    128-deep kTz stationary (start) + attn-bias accumulation via an
    identity-stationary matmul over the bf16 bias tile (stop; key-padding
    mask folded into the bias on host), then exp directly from PSUM (ACT)
    and attn@v with a fused ones-column denominator. k-proj for hp+1 and
    wave-A out-proj accumulation interleave as PE filler; kTz evacuation
    runs on DVE (tensor_scalar_add with the per-partition k bias).
  - per-hp normalization: denominator rows broadcast via bf16 selector
    matmuls, one DVE reciprocal on the broadcast, multiplies on the
    otherwise-idle GPSIMD engine (all-SBUF).
  - out-proj: wave A (2 tiles) accumulates during attention; wave B
    (6 tiles) runs et0..6 bursts through the final norm, then finishes.
"""
import sys

sys.path.insert(0, "/opt/trn_rl_repo")

import numpy as np
import ml_dtypes

import concourse.bass as bass
import concourse.bacc as bacc
import concourse.tile as tile
from concourse import mybir
from concourse.bass_utils import run_bass_kernel_spmd

F32 = mybir.dt.float32
BF16 = mybir.dt.bfloat16
Act = mybir.ActivationFunctionType
Alu = mybir.AluOpType
NPBF16 = ml_dtypes.bfloat16

B, T, S, E, H, D = 4, 1024, 1024, 1024, 16, 64
HP = H // 2          # head pairs
TS = T // 2          # per-core query rows (t-shard)
ET = E // 128        # 128-row tiles of the embed dim
SCALING = D ** -0.5
MASK_NEG = -10000.0

_CACHE = {}


def build_nc():
    nc = bacc.Bacc("TRN2", target_bir_lowering=False, debug=False, num_devices=8)

    qin_d = nc.dram_tensor("qin", [E, TS], BF16, kind="ExternalInput").ap()
    kin_d = nc.dram_tensor("kin", [E, S], BF16, kind="ExternalInput").ap()
    vin_d = nc.dram_tensor("vin", [E, S], BF16, kind="ExternalInput").ap()
    # bias with key-padding mask folded in, transposed to [H, S, TS], bf16
    bias_d = nc.dram_tensor("biasT", [H, S, TS], BF16, kind="ExternalInput").ap()
    wq_d = nc.dram_tensor("wqt", [E, E], BF16, kind="ExternalInput").ap()
    wk_d = nc.dram_tensor("wkt", [E, E], BF16, kind="ExternalInput").ap()
    wv_d = nc.dram_tensor("wvt", [E, E], BF16, kind="ExternalInput").ap()
    wo_d = nc.dram_tensor("wot", [E, E], BF16, kind="ExternalInput").ap()
    bq_d = nc.dram_tensor("bqs", [128, 8], F32, kind="ExternalInput").ap()
    bk_d = nc.dram_tensor("bks", [128, 8], F32, kind="ExternalInput").ap()
    bv_d = nc.dram_tensor("bvr", [128, E], F32, kind="ExternalInput").ap()
    bo_d = nc.dram_tensor("bor", [128, E], F32, kind="ExternalInput").ap()
    sel_d = nc.dram_tensor("sel2", [128, 128], BF16, kind="ExternalInput").ap()
    id_d = nc.dram_tensor("ident", [128, 128], BF16, kind="ExternalInput").ap()
    out_d = nc.dram_tensor("out", [TS, E], F32, kind="ExternalOutput").ap()

    with tile.TileContext(nc) as tc:
        with tc.tile_pool(name="consts", bufs=1) as consts, \
             tc.tile_pool(name="wpool", bufs=1) as wpool, \
             tc.tile_pool(name="kvin", bufs=1) as kvin, \
             tc.tile_pool(name="persist", bufs=1) as persist, \
             tc.tile_pool(name="estream", bufs=8) as estream, \
             tc.tile_pool(name="btp", bufs=8) as btp, \
             tc.tile_pool(name="rcpp", bufs=1) as rcpp, \
             tc.tile_pool(name="osbp", bufs=2) as osbp, \
             tc.tile_pool(name="psmain", bufs=3, space="PSUM") as psmain, \
             tc.tile_pool(name="psk", bufs=1, space="PSUM") as psk, \
             tc.tile_pool(name="psA", bufs=2, space="PSUM") as psAp, \
             tc.tile_pool(name="psot", bufs=1, space="PSUM") as psot:

            # ---- tiny constants first ----
            bq_sb = consts.tile([128, 8], F32, tag="bq")
            bk_sb = consts.tile([128, 8], F32, tag="bk")
            sel_sb = consts.tile([128, 128], BF16, tag="sel")
            id_sb = consts.tile([128, 128], BF16, tag="id")
            ones_col = consts.tile([128, 16], BF16, tag="ones_col")
            bvb = consts.tile([128, E], F32, tag="bvb")
            bob = consts.tile([128, E], F32, tag="bob")
            nc.vector.memset(ones_col, 1.0)

            # ---- input tiles: 2 et-chunks of 4 each (separate tiles so the
            # first chunk's consumers don't wait on the second DMA) ----
            def chunk_tiles(tag, width):
                return [wpool.tile([128, 4, width], BF16, tag=f"{tag}{c}",
                                   name=f"{tag}{c}") for c in range(2)]

            wq_c4 = [wpool.tile([128, 2, E], BF16, tag=f"wqf{c}",
                                name=f"wqf{c}") for c in range(4)]
            wk_c = chunk_tiles("wk", E)
            wo_c = chunk_tiles("wo", E)
            wv_c = [[wpool.tile([128, 4, 512], BF16, tag=f"wv{i}{c}",
                                name=f"wv{i}{c}") for c in range(2)]
                    for i in range(2)]
            qin_c4 = [kvin.tile([128, 2, TS], BF16, tag=f"qinf{c}",
                                name=f"qinf{c}") for c in range(4)]
            kin_c = [kvin.tile([128, 4, S], BF16, tag=f"kin{c}",
                               name=f"kin{c}") for c in range(2)]
            vin_c4 = [kvin.tile([128, 2, S], BF16, tag=f"vinf{c}",
                                name=f"vinf{c}") for c in range(4)]

            def w(tiles, et, colsl):
                return tiles[et // 4][:, et % 4, colsl]

            def load_chunk(dst, src, c, colsl=None):
                # DRAM rows [c*512:(c+1)*512] of src -> dst [128, 4, width]
                if colsl is None:
                    nc.sync.dma_start(
                        out=dst,
                        in_=src[c * 512:(c + 1) * 512, :].rearrange(
                            "(et p) x -> p et x", p=128))
                else:
                    nc.sync.dma_start(
                        out=dst,
                        in_=src[c * 512:(c + 1) * 512, colsl].rearrange(
                            "(et p) x -> p et x", p=128))

            def load_chunk2(dst, src, c):
                nc.sync.dma_start(
                    out=dst,
                    in_=src[c * 256:(c + 1) * 256, :].rearrange(
                        "(et p) x -> p et x", p=128))

            # startup-ordered DMAs: q deps, then v deps, then k deps
            for c in range(4):
                load_chunk2(wq_c4[c], wq_d, c)
                load_chunk2(qin_c4[c], qin_d, c)
            nc.sync.dma_start(out=bq_sb, in_=bq_d)
            nc.sync.dma_start(out=bk_sb, in_=bk_d)
            nc.sync.dma_start(out=bvb, in_=bv_d)

            def load_wv(i, c):
                nc.sync.dma_start(
                    out=wv_c[i][c],
                    in_=wv_d[c * 512:(c + 1) * 512, i * 512:(i + 1) * 512]
                    .rearrange("(et p) x -> p et x", p=128))

            load_wv(0, 0)
            load_chunk2(vin_c4[0], vin_d, 0)
            load_chunk2(vin_c4[1], vin_d, 1)
            load_wv(0, 1)
            load_chunk2(vin_c4[2], vin_d, 2)
            load_chunk2(vin_c4[3], vin_d, 3)
            load_wv(1, 0)
            load_wv(1, 1)
            nc.sync.dma_start(out=sel_sb, in_=sel_d)
            nc.sync.dma_start(out=id_sb, in_=id_d)
            load_chunk(wk_c[0], wk_d, 0)
            load_chunk(kin_c[0], kin_d, 0)
            load_chunk(wk_c[1], wk_d, 1)
            load_chunk(kin_c[1], kin_d, 1)

            # bias chunk stream: per (hp, hh) two chunks of 4 j-tiles
            bias_tiles = {}

            def load_bias_hp(hp):
                for hh in range(2):
                    h = 2 * hp + hh
                    for half in range(2):
                        t_ = btp.tile([128, 4, TS], BF16, tag="bt", name="bt")
                        nc.sync.dma_start(
                            out=t_,
                            in_=bias_d[h, half * 512:(half + 1) * 512, :]
                            .rearrange("(j p) t -> p j t", p=128))
                        bias_tiles[(hp, hh, half)] = t_

            load_bias_hp(0)
            load_bias_hp(1)
            nc.sync.dma_start(out=bob, in_=bo_d)
            load_chunk(wo_c[0], wo_d, 0)
            load_chunk(wo_c[1], wo_d, 1)

            # ---- persistent products ----
            qT = [persist.tile([128, TS], BF16, tag=f"qt{hp}", name=f"qt{hp}")
                  for hp in range(HP)]
            # zero-padded k tiles: kTz[(sh, hh)] has head hh's rows in
            # partitions hh*64..hh*64+63 and zeros elsewhere, so the score
            # matmul can use a full 128-deep stationary (accumulation-group
            # compatible with the identity bias matmul).
            kTz = {}
            for sh in range(2):
                for hh in range(2):
                    t_ = persist.tile([128, 512], BF16, tag=f"ktz{sh}{hh}",
                                      name=f"ktz{sh}{hh}")
                    nc.vector.memset(t_, 0.0)
                    kTz[(sh, hh)] = t_
            v65 = [persist.tile([128, H, 65], BF16, tag=f"v65_{j}",
                                name=f"v65_{j}")
                   for j in range(8)]
            otn2 = [persist.tile([128, TS], BF16, tag=f"otn2_{et}",
                                 name=f"otn2_{et}")
                    for et in range(ET)]

            # ---- q projection: qT[hp] = wqT.T @ queryT (+ bq, pre-scaled)
            # et-outer over 4-hp halves so the PE consumes each wq/qin chunk
            # as soon as its DMA lands ----
            for half in range(2):
                ps4 = [psmain.tile([128, 512], F32, tag="main", name="psm")
                       for _ in range(3)]
                ps4.append(psk.tile([128, 512], F32, tag="psk", name="psk"))
                for et in range(ET):
                    for i in range(4):
                        hp = half * 4 + i
                        nc.tensor.matmul(
                            ps4[i],
                            wq_c4[et // 2][:, et % 2, hp * 128:(hp + 1) * 128],
                            qin_c4[et // 2][:, et % 2, :],
                            start=(et == 0), stop=(et == ET - 1))
                for i in range(4):
                    hp = half * 4 + i
                    nc.scalar.activation(qT[hp], ps4[i], Act.Identity,
                                         bias=bq_sb[:, hp:hp + 1])

            # ---- v projection into v65 (s-major, per-head 65th col = 1) ----
            for j in range(8):
                nc.vector.tensor_copy(
                    out=v65[j][:, :, 64:65],
                    in_=ones_col.rearrange("p (h o) -> p h o", o=1))
            for ih in range(2):
                for sh in range(2):
                    for sp in range(2):
                        ps2 = [psmain.tile([128, 512], F32, tag="main",
                                           name="psm") for _ in range(2)]
                        for et in range(ET):
                            for i in range(2):
                                st = sp * 2 + i
                                nc.tensor.matmul(
                                    ps2[i],
                                    vin_c4[et // 2][:, et % 2,
                                                    sh * 512 + st * 128:
                                                    sh * 512 + (st + 1) * 128],
                                    wv_c[ih][et // 4][:, et % 4, :],
                                    start=(et == 0), stop=(et == ET - 1))
                        for i in range(2):
                            st = sp * 2 + i
                            j = sh * 4 + st
                            nc.vector.tensor_tensor(
                                out=v65[j][:, ih * 8:(ih + 1) * 8, 0:64],
                                in0=ps2[i].rearrange("p (h d) -> p h d", h=8),
                                in1=bvb[:, ih * 512:(ih + 1) * 512].rearrange(
                                    "p (h d) -> p h d", h=8),
                                op=Alu.add)

            # ---- k projection for head pair hp (inline or as PE filler) ----
            def k_proj(hp):
                items = []
                for sh in range(2):
                    ps_box = {}

                    def mm(et, sh=sh, ps_box=ps_box):
                        if et == 0:
                            ps_box["ps"] = psk.tile(
                                [128, 512], F32, tag="psk", name="psk")
                        nc.tensor.matmul(
                            ps_box["ps"],
                            w(wk_c, et, slice(hp * 128, (hp + 1) * 128)),
                            w(kin_c, et, slice(sh * 512, (sh + 1) * 512)),
                            start=(et == 0), stop=(et == ET - 1))
                        if et == ET - 1:
                            # evacuate into the zero-padded kTz tiles on DVE
                            for hh in range(2):
                                nc.vector.tensor_scalar_add(
                                    out=kTz[(sh, hh)][hh * 64:(hh + 1) * 64, :],
                                    in0=ps_box["ps"][hh * 64:(hh + 1) * 64, :],
                                    scalar1=bk_sb[hh * 64:(hh + 1) * 64,
                                                  hp:hp + 1])
                    items.extend([lambda et=et, mm=mm: mm(et)
                                  for et in range(ET)])
                return items

            # k-proj for hp 0 runs before attention
            for it in k_proj(0):
                it()

            # ---- wave-A out-proj accumulators: tiles (tt=0, ih=0/1) ----
            waveA = [(0, 0), (0, 1)]
            waveB = [(1, 0), (1, 1), (2, 0), (2, 1), (3, 0), (3, 1)]
            wa_ps = {}
            for tt, ih in waveA:
                wa_ps[(tt, ih)] = psAp.tile([128, 512], F32, tag="psA",
                                            name="psA")

            def wa_accum(hp):
                for tt, ih in waveA:
                    nc.tensor.matmul(
                        wa_ps[(tt, ih)],
                        otn2[hp][:, tt * 128:(tt + 1) * 128],
                        w(wo_c, hp, slice(ih * 512, (ih + 1) * 512)),
                        start=(hp == 0), stop=(hp == ET - 1))

            # ---- PE filler queue for the attention phase ----
            filler = []
            for hpx in range(1, HP):
                filler.extend(k_proj(hpx))

            def pull(n):
                for _ in range(n):
                    if filler:
                        filler.pop(0)()

            # ---- attention ----
            for hp in range(HP):
                poT = [psot.tile([65, 512], F32, tag=f"ot{hh}", name=f"ot{hh}")
                       for hh in range(2)]
                e_tiles = {}

                def score_pair(j, hp=hp, e_tiles=e_tiles):
                    sh, sl = j // 4, j % 4
                    pss = []
                    for hh in range(2):
                        pss.append(psmain.tile([128, 512], F32, tag="main",
                                               name="psm"))
                    for hh in range(2):
                        nc.tensor.matmul(
                            pss[hh],
                            kTz[(sh, hh)][:, sl * 128:(sl + 1) * 128],
                            qT[hp], start=True, stop=False)
                        bt = bias_tiles[(hp, hh, j // 4)]
                        nc.tensor.matmul(
                            pss[hh], id_sb, bt[:, j % 4, :],
                            start=False, stop=True)
                    pull(2)
                    for hh in range(2):
                        e_ = estream.tile([128, TS], BF16, tag="e", name="e")
                        nc.scalar.activation(e_, pss[hh], Act.Exp)
                        e_tiles[(j, hh)] = e_

                def o_mm(j, hh, hp=hp, poT=poT, e_tiles=e_tiles):
                    h = 2 * hp + hh
                    nc.tensor.matmul(poT[hh], v65[j][:, h, :],
                                     e_tiles.pop((j, hh)),
                                     start=(j == 0), stop=(j == 7))

                for j in range(8):
                    score_pair(j)
                    if j >= 3:
                        for hh in range(2):
                            o_mm(j - 3, hh)
                for jj in (5, 6, 7):
                    for hh in range(2):
                        o_mm(jj, hh)

                # prefetch bias for hp+2
                if hp + 2 < HP:
                    load_bias_hp(hp + 2)

                # ---- per-hp normalization ----
                den2 = rcpp.tile([128, 512], BF16, tag="den", name="den")
                ot_sb = rcpp.tile([128, 512], F32, tag="ots", name="ots")
                for hh in range(2):
                    nc.vector.tensor_copy(out=den2[64 * hh:64 * hh + 1, :],
                                          in_=poT[hh][64:65, :])
                    if hp < 7:
                        # evacuate early so poT's last reader is cheap (the
                        # next hp's o_mm WARs it); hp7 multiplies from PSUM
                        nc.vector.tensor_copy(
                            out=ot_sb[64 * hh:64 * (hh + 1), :],
                            in_=poT[hh][0:64, :])
                # wave-A accumulation for the previous hp covers the copy
                # latency (its otn2 is a full window old - no stall)
                if hp >= 1:
                    wa_accum(hp - 1)
                bc = psmain.tile([128, 512], F32, tag="main", name="psm")
                for hh in range(2):
                    # independent groups with disjoint 64-partition regions
                    nc.tensor.matmul(
                        bc[64 * hh:64 * (hh + 1), :],
                        sel_sb[64 * hh:64 * hh + 1, 64 * hh:64 * (hh + 1)],
                        den2[64 * hh:64 * hh + 1, :],
                        start=True, stop=True)
                bc_sb = rcpp.tile([128, 512], F32, tag="bcs", name="bcs")
                nc.vector.tensor_copy(out=bc_sb, in_=bc)
                rcp_sb = rcpp.tile([128, 512], F32, tag="rcps", name="rcps")
                nc.vector.reciprocal(out=rcp_sb, in_=bc_sb)
                for hh in range(2):
                    if hp < 7:
                        # all-SBUF multiply on the otherwise-idle GPSIMD
                        nc.gpsimd.tensor_tensor(
                            out=otn2[hp][hh * 64:(hh + 1) * 64, :],
                            in0=ot_sb[hh * 64:(hh + 1) * 64, :],
                            in1=rcp_sb[hh * 64:(hh + 1) * 64, :],
                            op=Alu.mult)
                    else:
                        nc.vector.tensor_tensor(
                            out=otn2[hp][hh * 64:(hh + 1) * 64, :],
                            in0=poT[hh][0:64, :],
                            in1=rcp_sb[hh * 64:(hh + 1) * 64, :],
                            op=Alu.mult)


            # ---- output projection tail ----
            # wave-B et0..6 bursts keep the PE hot while norm(7) finishes;
            # the et==7 matmul + evacuation follow once otn2[7] lands.
            wb_ps = {}

            def wb_burst(tt, ih, pool=None):
                wb_ps[(tt, ih)] = (pool or psmain).tile(
                    [128, 512], F32,
                    tag="main" if pool is None else "psk", name="psm")
                for et in range(ET - 1):
                    nc.tensor.matmul(
                        wb_ps[(tt, ih)],
                        otn2[et][:, tt * 128:(tt + 1) * 128],
                        w(wo_c, et, slice(ih * 512, (ih + 1) * 512)),
                        start=(et == 0), stop=False)

            def evac(ps, tt, ih):
                o = osbp.tile([128, 512], F32, tag="osb", name="osb")
                nc.vector.tensor_tensor(
                    out=o, in0=ps, in1=bob[:, ih * 512:(ih + 1) * 512],
                    op=Alu.add)
                nc.sync.dma_start(
                    out=out_d[tt * 128:(tt + 1) * 128,
                              ih * 512:(ih + 1) * 512],
                    in_=o)

            def wb_finish(tt, ih):
                nc.tensor.matmul(
                    wb_ps[(tt, ih)],
                    otn2[7][:, tt * 128:(tt + 1) * 128],
                    w(wo_c, 7, slice(ih * 512, (ih + 1) * 512)),
                    start=False, stop=True)
                evac(wb_ps[(tt, ih)], tt, ih)

            g1, g2 = waveB[:4], waveB[4:]
            for i, (tt, ih) in enumerate(g1):
                wb_burst(tt, ih, pool=psk if i == 3 else None)
            # final wave-A accumulation (otn2[7] lands during the bursts)
            wa_accum(7)
            while filler:
                filler.pop(0)()
            for tt, ih in waveA:
                evac(wa_ps[(tt, ih)], tt, ih)
            for tt, ih in g1:
                wb_finish(tt, ih)
            for tt, ih in g2:
                wb_burst(tt, ih)
            for tt, ih in g2:
                wb_finish(tt, ih)

    nc.compile()
    return nc


def _prepare_in_maps(query, key, value, key_padding_mask, attn_bias,
                     wq, bq, wk, bk, wv, bv, wo, bo):
    wqt = (np.ascontiguousarray(wq.T) * SCALING).astype(NPBF16)
    wkt = np.ascontiguousarray(wk.T).astype(NPBF16)
    wvt = np.ascontiguousarray(wv.T).astype(NPBF16)
    wot = np.ascontiguousarray(wo.T).astype(NPBF16)
    bqs = np.ascontiguousarray((bq * SCALING).reshape(8, 128).T)
    bks = np.ascontiguousarray(bk.astype(np.float32).reshape(8, 128).T)
    bvr = np.ascontiguousarray(np.broadcast_to(
        np.asarray(bv, np.float32)[None, :], (128, E)))
    bor = np.ascontiguousarray(np.broadcast_to(
        np.asarray(bo, np.float32)[None, :], (128, E)))
    sel2 = np.zeros((128, 128), NPBF16)
    sel2[0, :64] = 1.0
    sel2[64, 64:] = 1.0
    ident = np.eye(128, dtype=NPBF16)

    kin_b = [np.ascontiguousarray(key[b_].T).astype(NPBF16) for b_ in range(B)]
    vin_b = [np.ascontiguousarray(value[b_].T).astype(NPBF16) for b_ in range(B)]
    masked = [np.where(key_padding_mask[b_], np.float32(MASK_NEG),
                       np.float32(0.0)) for b_ in range(B)]

    in_maps = []
    for c in range(8):
        b_, th = c // 2, c % 2
        qin = np.ascontiguousarray(
            query[b_, th * TS:(th + 1) * TS, :].T).astype(NPBF16)
        biasT = (attn_bias[b_ * H:(b_ + 1) * H, th * TS:(th + 1) * TS, :]
                 .transpose(0, 2, 1) + masked[b_][None, :, None])
        biasT = np.ascontiguousarray(biasT).astype(NPBF16)
        in_maps.append({
            "qin": qin, "kin": kin_b[b_], "vin": vin_b[b_],
            "biasT": biasT,
            "wqt": wqt, "wkt": wkt, "wvt": wvt, "wot": wot,
            "bqs": bqs, "bks": bks, "bvr": bvr, "bor": bor,
            "sel2": sel2, "ident": ident,
        })
    return in_maps


def kernel(query, key, value, key_padding_mask, attn_bias,
           wq, bq, wk, bk, wv, bv, wo, bo, _run_kwargs=None):
    query = np.asarray(query, dtype=np.float32)
    key = np.asarray(key, dtype=np.float32)
    value = np.asarray(value, dtype=np.float32)
    key_padding_mask = np.asarray(key_padding_mask)
    attn_bias = np.asarray(attn_bias, dtype=np.float32)
    wq, bq = np.asarray(wq, np.float32), np.asarray(bq, np.float32)
    wk, bk = np.asarray(wk, np.float32), np.asarray(bk, np.float32)
    wv, bv = np.asarray(wv, np.float32), np.asarray(bv, np.float32)
    wo, bo = np.asarray(wo, np.float32), np.asarray(bo, np.float32)

    if "nc" not in _CACHE:
        _CACHE["nc"] = build_nc()
    nc = _CACHE["nc"]

    in_maps = _prepare_in_maps(query, key, value, key_padding_mask, attn_bias,
                               wq, bq, wk, bk, wv, bv, wo, bo)
    res = run_bass_kernel_spmd(nc, in_maps, core_ids=list(range(8)),
                               **(_run_kwargs or {}))
    _CACHE["last_results"] = res

    out = np.empty((B, T, E), dtype=np.float32)
    for c in range(8):
        b_, th = c // 2, c % 2
        out[b_, th * TS:(th + 1) * TS, :] = res.results[c]["out"]
    return out


# revision 45
# speedup vs baseline: 1.0473x; 1.0392x over previous
"""Trainium2 Bass kernel for CrossMultiheadAttention.

B=4, T=S=1024, E=1024, H=16, D=64. 8 NeuronCores.

Sharding: core c handles (batch b=c//2, T-half th=c%2) -> 512 query rows.
Each core computes k/v projections for its whole batch (duplicated between
the 2 cores sharing a batch), all 16 heads of attention for its queries and
the full output projection for its rows. Output gather is a pure concat.

Schedule (v4): PE-centric pipeline that keeps the tensor engine streaming
continuously (p-state stays at max clock):
  - merged multi-tile DMAs, ordered wq/qin -> wv/vin -> wk/kin -> bias/wo
  - pre-attention: q-proj, v-proj (both halves), k-proj head-pair 0
  - attention hp=0..7 per (j, head): score matmul over a zero-padded
    128-deep kTz stationary (start) + attn-bias accumulation via an
    identity-stationary matmul over the bf16 bias tile (stop; key-padding
    mask folded into the bias on host), then exp directly from PSUM (ACT)
    and attn@v with a fused ones-column denominator. k-proj for hp+1 and
    wave-A out-proj accumulation interleave as PE filler; kTz evacuation
    runs on DVE (tensor_scalar_add with the per-partition k bias).
  - per-hp normalization: denominator rows broadcast via bf16 selector
    matmuls, one DVE reciprocal on the broadcast, multiplies on the
    otherwise-idle GPSIMD engine (all-SBUF).
  - out-proj: wave A (2 tiles) accumulates during attention; wave B
    (6 tiles) runs et0..6 bursts through the final norm, then finishes.
"""
import sys

sys.path.insert(0, "/opt/trn_rl_repo")

import numpy as np
import ml_dtypes

import concourse.bass as bass
import concourse.bacc as bacc
import concourse.tile as tile
from concourse import mybir
from concourse.bass_utils import run_bass_kernel_spmd


def _pbcast(ap, nparts):
    """View a [1, N] row replicated across nparts partitions via a
    0-stride partition dim - DMA-source only."""
    return bass.AP(tensor=ap.tensor, offset=ap.offset,
                   ap=[[0, nparts]] + [list(d) for d in ap.ap[1:]])


F32 = mybir.dt.float32
BF16 = mybir.dt.bfloat16
Act = mybir.ActivationFunctionType
Alu = mybir.AluOpType
NPBF16 = ml_dtypes.bfloat16

B, T, S, E, H, D = 4, 1024, 1024, 1024, 16, 64
HP = H // 2          # head pairs
TS = T // 2          # per-core query rows (t-shard)
ET = E // 128        # 128-row tiles of the embed dim
SCALING = D ** -0.5
MASK_NEG = -10000.0

_CACHE = {}


def build_nc():
    nc = bacc.Bacc("TRN2", target_bir_lowering=False, debug=False, num_devices=8)

    qin_d = nc.dram_tensor("qin", [E, TS], BF16, kind="ExternalInput").ap()
    kin_d = nc.dram_tensor("kin", [E, S], BF16, kind="ExternalInput").ap()
    vin_d = nc.dram_tensor("vin", [E, S], BF16, kind="ExternalInput").ap()
    # bias with key-padding mask folded in, transposed to [H, S, TS], bf16
    bias_d = nc.dram_tensor("biasT", [H, S, TS], BF16, kind="ExternalInput").ap()
    wq_d = nc.dram_tensor("wqt", [E, E], BF16, kind="ExternalInput").ap()
    wk_d = nc.dram_tensor("wkt", [E, E], BF16, kind="ExternalInput").ap()
    wv_d = nc.dram_tensor("wvt", [E, E], BF16, kind="ExternalInput").ap()
    wo_d = nc.dram_tensor("wot", [E, E], BF16, kind="ExternalInput").ap()
    bq_d = nc.dram_tensor("bqs", [128, 8], F32, kind="ExternalInput").ap()
    bk_d = nc.dram_tensor("bks", [128, 8], F32, kind="ExternalInput").ap()
    bv_d = nc.dram_tensor("bvr", [128, E], F32, kind="ExternalInput").ap()
    bo_d = nc.dram_tensor("bor", [128, E], F32, kind="ExternalInput").ap()
    sel_d = nc.dram_tensor("sel2", [128, 128], BF16, kind="ExternalInput").ap()
    id_d = nc.dram_tensor("ident", [128, 128], BF16, kind="ExternalInput").ap()
    out_d = nc.dram_tensor("out", [TS, E], F32, kind="ExternalOutput").ap()

    with tile.TileContext(nc) as tc:
        with tc.tile_pool(name="consts", bufs=1) as consts, \
             tc.tile_pool(name="wpool", bufs=1) as wpool, \
             tc.tile_pool(name="kvin", bufs=1) as kvin, \
             tc.tile_pool(name="persist", bufs=1) as persist, \
             tc.tile_pool(name="estream", bufs=6) as estream, \
             tc.tile_pool(name="btp", bufs=8) as btp, \
             tc.tile_pool(name="rcpp", bufs=1) as rcpp, \
             tc.tile_pool(name="osbp", bufs=2) as osbp, \
             tc.tile_pool(name="dramp", bufs=2, space="DRAM") as dramp, \
             tc.tile_pool(name="psmain", bufs=3, space="PSUM") as psmain, \
             tc.tile_pool(name="psk", bufs=1, space="PSUM") as psk, \
             tc.tile_pool(name="psA", bufs=2, space="PSUM") as psAp, \
             tc.tile_pool(name="psot", bufs=1, space="PSUM") as psot:

            # ---- tiny constants first ----
            bq_sb = consts.tile([128, 8], F32, tag="bq")
            bk_sb = consts.tile([128, 8], F32, tag="bk")
            sel_sb = consts.tile([128, 128], BF16, tag="sel")
            id_sb = consts.tile([128, 128], BF16, tag="id")
            ones_col = consts.tile([128, 16], BF16, tag="ones_col")
            bvb = consts.tile([128, E], F32, tag="bvb")
            bob = consts.tile([128, E], F32, tag="bob")
            nc.vector.memset(ones_col, 1.0)

            # ---- input tiles: 2 et-chunks of 4 each (separate tiles so the
            # first chunk's consumers don't wait on the second DMA) ----
            def chunk_tiles(tag, width):
                return [wpool.tile([128, 4, width], BF16, tag=f"{tag}{c}",
                                   name=f"{tag}{c}") for c in range(2)]

            wq_c4 = [wpool.tile([128, 2, E], BF16, tag=f"wqf{c}",
                                name=f"wqf{c}") for c in range(4)]
            wk_c = chunk_tiles("wk", E)
            wo_c = chunk_tiles("wo", E)
            wv_c = [[wpool.tile([128, 4, 512], BF16, tag=f"wv{i}{c}",
                                name=f"wv{i}{c}") for c in range(2)]
                    for i in range(2)]
            qin_c4 = [kvin.tile([128, 2, TS], BF16, tag=f"qinf{c}",
                                name=f"qinf{c}") for c in range(4)]
            kin_c = [kvin.tile([128, 4, S], BF16, tag=f"kin{c}",
                               name=f"kin{c}") for c in range(2)]
            vin_c4 = [kvin.tile([128, 2, S], BF16, tag=f"vinf{c}",
                                name=f"vinf{c}") for c in range(4)]

            def w(tiles, et, colsl):
                return tiles[et // 4][:, et % 4, colsl]

            def load_chunk(dst, src, c, colsl=None):
                # DRAM rows [c*512:(c+1)*512] of src -> dst [128, 4, width]
                if colsl is None:
                    nc.sync.dma_start(
                        out=dst,
                        in_=src[c * 512:(c + 1) * 512, :].rearrange(
                            "(et p) x -> p et x", p=128))
                else:
                    nc.sync.dma_start(
                        out=dst,
                        in_=src[c * 512:(c + 1) * 512, colsl].rearrange(
                            "(et p) x -> p et x", p=128))

            def load_chunk2(dst, src, c):
                nc.sync.dma_start(
                    out=dst,
                    in_=src[c * 256:(c + 1) * 256, :].rearrange(
                        "(et p) x -> p et x", p=128))

            # startup-ordered DMAs: q deps, then v deps, then k deps
            for c in range(4):
                load_chunk2(wq_c4[c], wq_d, c)
                load_chunk2(qin_c4[c], qin_d, c)
            nc.sync.dma_start(out=bq_sb, in_=bq_d)
            nc.sync.dma_start(out=bk_sb, in_=bk_d)
            nc.sync.dma_start(out=bvb, in_=bv_d)

            def load_wv(i, c):
                nc.sync.dma_start(
                    out=wv_c[i][c],
                    in_=wv_d[c * 512:(c + 1) * 512, i * 512:(i + 1) * 512]
                    .rearrange("(et p) x -> p et x", p=128))

            load_wv(0, 0)
            load_chunk2(vin_c4[0], vin_d, 0)
            load_chunk2(vin_c4[1], vin_d, 1)
            load_wv(0, 1)
            load_chunk2(vin_c4[2], vin_d, 2)
            load_chunk2(vin_c4[3], vin_d, 3)
            load_wv(1, 0)
            load_wv(1, 1)
            nc.sync.dma_start(out=sel_sb, in_=sel_d)
            nc.sync.dma_start(out=id_sb, in_=id_d)
            load_chunk(wk_c[0], wk_d, 0)
            load_chunk(kin_c[0], kin_d, 0)
            load_chunk(wk_c[1], wk_d, 1)
            load_chunk(kin_c[1], kin_d, 1)

            # bias chunk stream: per (hp, hh) two chunks of 4 j-tiles
            bias_tiles = {}

            def load_bias_hp(hp):
                for hh in range(2):
                    h = 2 * hp + hh
                    for half in range(2):
                        t_ = btp.tile([128, 4, TS], BF16, tag="bt", name="bt")
                        nc.sync.dma_start(
                            out=t_,
                            in_=bias_d[h, half * 512:(half + 1) * 512, :]
                            .rearrange("(j p) t -> p j t", p=128))
                        bias_tiles[(hp, hh, half)] = t_

            load_bias_hp(0)
            load_bias_hp(1)
            nc.sync.dma_start(out=bob, in_=bo_d)
            load_chunk(wo_c[0], wo_d, 0)
            load_chunk(wo_c[1], wo_d, 1)

            # ---- persistent products ----
            qT = [persist.tile([128, TS], BF16, tag=f"qt{hp}", name=f"qt{hp}")
                  for hp in range(HP)]
            # zero-padded k tiles: kTz[(sh, hh)] has head hh's rows in
            # partitions hh*64..hh*64+63 and zeros elsewhere, so the score
            # matmul can use a full 128-deep stationary (accumulation-group
            # compatible with the identity bias matmul).
            kTz = {}
            for sh in range(2):
                for hh in range(2):
                    t_ = persist.tile([128, 512], BF16, tag=f"ktz{sh}{hh}",
                                      name=f"ktz{sh}{hh}")
                    nc.vector.memset(t_, 0.0)
                    kTz[(sh, hh)] = t_
            v65 = [persist.tile([128, H, 65], BF16, tag=f"v65_{j}",
                                name=f"v65_{j}")
                   for j in range(8)]
            otn2 = [persist.tile([128, TS], BF16, tag=f"otn2_{et}",
                                 name=f"otn2_{et}")
                    for et in range(ET)]

            # ---- q projection: qT[hp] = wqT.T @ queryT (+ bq, pre-scaled)
            # et-outer over 4-hp halves so the PE consumes each wq/qin chunk
            # as soon as its DMA lands ----
            for half in range(2):
                ps4 = [psmain.tile([128, 512], F32, tag="main", name="psm")
                       for _ in range(3)]
                ps4.append(psk.tile([128, 512], F32, tag="psk", name="psk"))
                for et in range(ET):
                    for i in range(4):
                        hp = half * 4 + i
                        nc.tensor.matmul(
                            ps4[i],
                            wq_c4[et // 2][:, et % 2, hp * 128:(hp + 1) * 128],
                            qin_c4[et // 2][:, et % 2, :],
                            start=(et == 0), stop=(et == ET - 1))
                for i in range(4):
                    hp = half * 4 + i
                    nc.scalar.activation(qT[hp], ps4[i], Act.Identity,
                                         bias=bq_sb[:, hp:hp + 1])

            # ---- v projection into v65 (s-major, per-head 65th col = 1) ----
            for j in range(8):
                nc.vector.tensor_copy(
                    out=v65[j][:, :, 64:65],
                    in_=ones_col.rearrange("p (h o) -> p h o", o=1))
            for ih in range(2):
                for sh in range(2):
                    for sp in range(2):
                        ps2 = [psmain.tile([128, 512], F32, tag="main",
                                           name="psm") for _ in range(2)]
                        for et in range(ET):
                            for i in range(2):
                                st = sp * 2 + i
                                nc.tensor.matmul(
                                    ps2[i],
                                    vin_c4[et // 2][:, et % 2,
                                                    sh * 512 + st * 128:
                                                    sh * 512 + (st + 1) * 128],
                                    wv_c[ih][et // 4][:, et % 4, :],
                                    start=(et == 0), stop=(et == ET - 1))
                        for i in range(2):
                            st = sp * 2 + i
                            j = sh * 4 + st
                            nc.vector.tensor_tensor(
                                out=v65[j][:, ih * 8:(ih + 1) * 8, 0:64],
                                in0=ps2[i].rearrange("p (h d) -> p h d", h=8),
                                in1=bvb[:, ih * 512:(ih + 1) * 512].rearrange(
                                    "p (h d) -> p h d", h=8),
                                op=Alu.add)

            # ---- k projection for head pair hp (inline or as PE filler) ----
            def k_proj(hp):
                items = []
                for sh in range(2):
                    ps_box = {}

                    def mm(et, sh=sh, ps_box=ps_box):
                        if et == 0:
                            ps_box["ps"] = psk.tile(
                                [128, 512], F32, tag="psk", name="psk")
                        nc.tensor.matmul(
                            ps_box["ps"],
                            w(wk_c, et, slice(hp * 128, (hp + 1) * 128)),
                            w(kin_c, et, slice(sh * 512, (sh + 1) * 512)),
                            start=(et == 0), stop=(et == ET - 1))
                        if et == ET - 1:
                            # evacuate into the zero-padded kTz tiles on DVE
                            for hh in range(2):
                                nc.vector.tensor_scalar_add(
                                    out=kTz[(sh, hh)][hh * 64:(hh + 1) * 64, :],
                                    in0=ps_box["ps"][hh * 64:(hh + 1) * 64, :],
                                    scalar1=bk_sb[hh * 64:(hh + 1) * 64,
                                                  hp:hp + 1])
                    items.extend([lambda et=et, mm=mm: mm(et)
                                  for et in range(ET)])
                return items

            # k-proj for hp 0 runs before attention
            for it in k_proj(0):
                it()

            # ---- wave-A out-proj accumulators: tiles (tt=0, ih=0/1) ----
            waveA = [(0, 0), (0, 1)]
            waveB = [(1, 0), (1, 1), (2, 0), (2, 1), (3, 0), (3, 1)]
            wa_ps = {}
            for tt, ih in waveA:
                wa_ps[(tt, ih)] = psAp.tile([128, 512], F32, tag="psA",
                                            name="psA")

            def wa_accum(hp):
                for tt, ih in waveA:
                    nc.tensor.matmul(
                        wa_ps[(tt, ih)],
                        otn2[hp][:, tt * 128:(tt + 1) * 128],
                        w(wo_c, hp, slice(ih * 512, (ih + 1) * 512)),
                        start=(hp == 0), stop=(hp == ET - 1))

            # ---- PE filler queue for the attention phase ----
            filler = []
            for hpx in range(1, HP):
                filler.extend(k_proj(hpx))

            def pull(n):
                for _ in range(n):
                    if filler:
                        filler.pop(0)()

            # ---- attention ----
            for hp in range(HP):
                poT = [psot.tile([65, 512], F32, tag=f"ot{hh}", name=f"ot{hh}")
                       for hh in range(2)]
                e_tiles = {}

                def score_pair(j, hp=hp, e_tiles=e_tiles):
                    sh, sl = j // 4, j % 4
                    pss = []
                    for hh in range(2):
                        pss.append(psmain.tile([128, 512], F32, tag="main",
                                               name="psm"))
                    for hh in range(2):
                        nc.tensor.matmul(
                            pss[hh],
                            kTz[(sh, hh)][:, sl * 128:(sl + 1) * 128],
                            qT[hp], start=True, stop=False)
                        bt = bias_tiles[(hp, hh, j // 4)]
                        nc.tensor.matmul(
                            pss[hh], id_sb, bt[:, j % 4, :],
                            start=False, stop=True)
                    pull(2)
                    for hh in range(2):
                        e_ = estream.tile([128, TS], BF16, tag="e", name="e")
                        nc.scalar.activation(e_, pss[hh], Act.Exp)
                        e_tiles[(j, hh)] = e_

                def o_mm(j, hh, hp=hp, poT=poT, e_tiles=e_tiles):
                    h = 2 * hp + hh
                    nc.tensor.matmul(poT[hh], v65[j][:, h, :],
                                     e_tiles.pop((j, hh)),
                                     start=(j == 0), stop=(j == 7))

                for j in range(8):
                    score_pair(j)
                    if j >= 2:
                        for hh in range(2):
                            o_mm(j - 2, hh)
                for jj in (6, 7):
                    for hh in range(2):
                        o_mm(jj, hh)

                # prefetch bias for hp+2
                if hp + 2 < HP:
                    load_bias_hp(hp + 2)

                # ---- per-hp normalization ----
                den2 = rcpp.tile([128, 512], BF16, tag="den", name="den")
                ot_sb = rcpp.tile([128, 512], F32, tag="ots", name="ots")
                denf = rcpp.tile([128, 512], F32, tag="denf", name="denf")
                for hh in range(2):
                    nc.vector.tensor_copy(
                        out=(denf if hp < 7 else den2)[64 * hh:64 * hh + 1, :],
                        in_=poT[hh][64:65, :])
                    if hp < 7:
                        # evacuate early so poT's last reader is cheap (the
                        # next hp's o_mm WARs it); hp7 multiplies from PSUM
                        nc.vector.tensor_copy(
                            out=ot_sb[64 * hh:64 * (hh + 1), :],
                            in_=poT[hh][0:64, :])
                # wave-A accumulation for the previous hp covers the copy
                # latency (its otn2 is a full window old - no stall)
                if hp >= 1:
                    wa_accum(hp - 1)
                bc_sb = rcpp.tile([128, 512], F32, tag="bcs", name="bcs")
                if hp < 7:
                    # broadcast via DMA round-trip (off the PE-bound window;
                    # the chain is consumed a full window later)
                    dend = dramp.tile([2, 512], F32, tag="dend", name="dend")
                    for hh in range(2):
                        nc.sync.dma_start(
                            out=dend[hh:hh + 1, :],
                            in_=denf[64 * hh:64 * hh + 1, :])
                    for hh in range(2):
                        nc.sync.dma_start(
                            out=bc_sb[64 * hh:64 * (hh + 1), :],
                            in_=_pbcast(dend[hh:hh + 1, :], 64))
                else:
                    # low-latency PE path for the tail-critical last hp
                    bc = psmain.tile([128, 512], F32, tag="main", name="psm")
                    for hh in range(2):
                        nc.tensor.matmul(
                            bc[64 * hh:64 * (hh + 1), :],
                            sel_sb[64 * hh:64 * hh + 1, 64 * hh:64 * (hh + 1)],
                            den2[64 * hh:64 * hh + 1, :],
                            start=True, stop=True)
                    nc.vector.tensor_copy(out=bc_sb, in_=bc)
                rcp_sb = rcpp.tile([128, 512], F32, tag="rcps", name="rcps")
                nc.vector.reciprocal(out=rcp_sb, in_=bc_sb)
                for hh in range(2):
                    if hp < 7:
                        # all-SBUF multiply on the otherwise-idle GPSIMD
                        nc.gpsimd.tensor_tensor(
                            out=otn2[hp][hh * 64:(hh + 1) * 64, :],
                            in0=ot_sb[hh * 64:(hh + 1) * 64, :],
                            in1=rcp_sb[hh * 64:(hh + 1) * 64, :],
                            op=Alu.mult)
                    else:
                        nc.vector.tensor_tensor(
                            out=otn2[hp][hh * 64:(hh + 1) * 64, :],
                            in0=poT[hh][0:64, :],
                            in1=rcp_sb[hh * 64:(hh + 1) * 64, :],
                            op=Alu.mult)


            # ---- output projection tail ----
            # wave-B et0..6 bursts keep the PE hot while norm(7) finishes;
            # the et==7 matmul + evacuation follow once otn2[7] lands.
            wb_ps = {}

            def wb_burst(tt, ih, pool=None):
                wb_ps[(tt, ih)] = (pool or psmain).tile(
                    [128, 512], F32,
                    tag="main" if pool is None else "psk", name="psm")
                for et in range(ET - 1):
                    nc.tensor.matmul(
                        wb_ps[(tt, ih)],
                        otn2[et][:, tt * 128:(tt + 1) * 128],
                        w(wo_c, et, slice(ih * 512, (ih + 1) * 512)),
                        start=(et == 0), stop=False)

            def evac(ps, tt, ih):
                o = osbp.tile([128, 512], F32, tag="osb", name="osb")
                nc.vector.tensor_tensor(
                    out=o, in0=ps, in1=bob[:, ih * 512:(ih + 1) * 512],
                    op=Alu.add)
                nc.sync.dma_start(
                    out=out_d[tt * 128:(tt + 1) * 128,
                              ih * 512:(ih + 1) * 512],
                    in_=o)

            def wb_finish(tt, ih):
                nc.tensor.matmul(
                    wb_ps[(tt, ih)],
                    otn2[7][:, tt * 128:(tt + 1) * 128],
                    w(wo_c, 7, slice(ih * 512, (ih + 1) * 512)),
                    start=False, stop=True)
                evac(wb_ps[(tt, ih)], tt, ih)

            g1, g2 = waveB[:4], waveB[4:]
            for i, (tt, ih) in enumerate(g1):
                wb_burst(tt, ih, pool=psk if i == 3 else None)
            # final wave-A accumulation (otn2[7] lands during the bursts)
            wa_accum(7)
            while filler:
                filler.pop(0)()
            for tt, ih in waveA:
                evac(wa_ps[(tt, ih)], tt, ih)
            for tt, ih in g1:
                wb_finish(tt, ih)
            for tt, ih in g2:
                wb_burst(tt, ih)
            for tt, ih in g2:
                wb_finish(tt, ih)

    nc.compile()
    return nc


def _prepare_in_maps(query, key, value, key_padding_mask, attn_bias,
                     wq, bq, wk, bk, wv, bv, wo, bo):
    wqt = (np.ascontiguousarray(wq.T) * SCALING).astype(NPBF16)
    wkt = np.ascontiguousarray(wk.T).astype(NPBF16)
    wvt = np.ascontiguousarray(wv.T).astype(NPBF16)
    wot = np.ascontiguousarray(wo.T).astype(NPBF16)
    bqs = np.ascontiguousarray((bq * SCALING).reshape(8, 128).T)
    bks = np.ascontiguousarray(bk.astype(np.float32).reshape(8, 128).T)
    bvr = np.ascontiguousarray(np.broadcast_to(
        np.asarray(bv, np.float32)[None, :], (128, E)))
    bor = np.ascontiguousarray(np.broadcast_to(
        np.asarray(bo, np.float32)[None, :], (128, E)))
    sel2 = np.zeros((128, 128), NPBF16)
    sel2[0, :64] = 1.0
    sel2[64, 64:] = 1.0
    ident = np.eye(128, dtype=NPBF16)

    kin_b = [np.ascontiguousarray(key[b_].T).astype(NPBF16) for b_ in range(B)]
    vin_b = [np.ascontiguousarray(value[b_].T).astype(NPBF16) for b_ in range(B)]
    masked = [np.where(key_padding_mask[b_], np.float32(MASK_NEG),
                       np.float32(0.0)) for b_ in range(B)]

    in_maps = []
    for c in range(8):
        b_, th = c // 2, c % 2
        qin = np.ascontiguousarray(
            query[b_, th * TS:(th + 1) * TS, :].T).astype(NPBF16)
        biasT = (attn_bias[b_ * H:(b_ + 1) * H, th * TS:(th + 1) * TS, :]
                 .transpose(0, 2, 1) + masked[b_][None, :, None])
        biasT = np.ascontiguousarray(biasT).astype(NPBF16)
        in_maps.append({
            "qin": qin, "kin": kin_b[b_], "vin": vin_b[b_],
            "biasT": biasT,
            "wqt": wqt, "wkt": wkt, "wvt": wvt, "wot": wot,
            "bqs": bqs, "bks": bks, "bvr": bvr, "bor": bor,
            "sel2": sel2, "ident": ident,
        })
    return in_maps


def kernel(query, key, value, key_padding_mask, attn_bias,
           wq, bq, wk, bk, wv, bv, wo, bo, _run_kwargs=None):
    query = np.asarray(query, dtype=np.float32)
    key = np.asarray(key, dtype=np.float32)
    value = np.asarray(value, dtype=np.float32)
    key_padding_mask = np.asarray(key_padding_mask)
    attn_bias = np.asarray(attn_bias, dtype=np.float32)
    wq, bq = np.asarray(wq, np.float32), np.asarray(bq, np.float32)
    wk, bk = np.asarray(wk, np.float32), np.asarray(bk, np.float32)
    wv, bv = np.asarray(wv, np.float32), np.asarray(bv, np.float32)
    wo, bo = np.asarray(wo, np.float32), np.asarray(bo, np.float32)

    if "nc" not in _CACHE:
        _CACHE["nc"] = build_nc()
    nc = _CACHE["nc"]

    in_maps = _prepare_in_maps(query, key, value, key_padding_mask, attn_bias,
                               wq, bq, wk, bk, wv, bv, wo, bo)
    res = run_bass_kernel_spmd(nc, in_maps, core_ids=list(range(8)),
                               **(_run_kwargs or {}))
    _CACHE["last_results"] = res

    out = np.empty((B, T, E), dtype=np.float32)
    for c in range(8):
        b_, th = c // 2, c % 2
        out[b_, th * TS:(th + 1) * TS, :] = res.results[c]["out"]
    return out
